# revision 1
# baseline (speedup 1.0000x reference)
"""Trainium2 Bass kernel for nn_Attention2 (dense transformer block with
softmax over the heads axis).

Computation per (n, t) batch b (B = n*t = 4096 total, X_b = x[n,:,t,:].T is
[vv=25, c=512]):
    qkv = X_b @ w_qkv.T, split into q,k,v heads [h=8, 25, hd=64]
    s[h,i,j] = (q[h,i,:] . k[h,j,:]) / 8      (scale folded into w_q on host)
    p = softmax over h (axis 0)
    o[h,i,:] = sum_j p[h,i,j] v[h,j,:]  -> [25, 512] -> @ w_proj.T
    out[n,:,t,:] = result.T

Sharding: data-parallel over n, 2 n-values (512 batches) per core, 8 cores.

Layout trick: x[n, :, t, :] is naturally X_b^T ([c, vv], c on partitions), so
the qkv and proj matmuls run as weight-stationary batched GEMMs with the
batch dim fused into the moving free dim (16 batches -> N=400).  v is
computed in V[j, c'] layout (j on partitions) via x-slab-stationary matmuls
so the attention-core matmuls need no transposes.  fp32r (1 cycle/row vs 4
for fp32, ~1.6e-4 rel err) is used for the three big GEMMs; the tiny
attention matmuls run fp32 packed onto the PE array with tile_position.
"""
import numpy as np
import concourse.bass as bass
import concourse.mybir as mybir
import concourse.tile as tile
from concourse.bass_utils import run_bass_kernel_spmd
from concourse.vector_clock import ScopedClock, VectorClock

F32 = mybir.dt.float32
F32R = mybir.dt.float32r
F16 = mybir.dt.float16

N_CORES = 8
NN_PER_CORE = 2        # n values per core
T = 256
VV = 25
C = 512
H = 8
HD = 64
TG = 16                # t values (batches) per group
NGROUPS = NN_PER_CORE * (T // TG)   # 32 groups per core
NB = TG * VV           # 400 moving columns per group


def _split_drain_and_barrier(self, tick_clock, wait_clock):
    # walrus caps sync-wait commands at 1 for CTRL_NO; split the kernel-tail
    # drain into one drain per pending proc.
    vc = tick_clock.global_clock
    n = len(vc)
    for i in range(n):
        if vc[i] == 0:
            continue
        sub = VectorClock([vc[j] if j == i else 0 for j in range(n)])
        d = self.nc.sync.drain()
        wait_clock.add_sem_waits(d.ins, ScopedClock({None: sub}))
    self.nc.all_engine_barrier()
    assert self.sems is not None
    popped = self.nc._tile_sem_poison_stack.pop()
    assert popped is self._sem_poison
    self.nc.clear_and_free_semaphores(list(self.sems.allocated().values()))
    self.nc.all_engine_barrier()


tile.TileContext._drain_and_barrier = _split_drain_and_barrier


def split_excess_waits(nc, limit=1):
    """walrus codegen allows very few sync-wait commands per instruction
    (1 for matmul/drain/DMA structs).  Move excess waits onto same-engine
    NoOp carriers inserted just before the instruction — same semantics,
    since each engine executes its queue in order."""
    k = 0
    for fn in nc.m.functions:
        for bb in fn.blocks:
            out = []
            for ins in bb.instructions:
                si = ins.sync_info
                waits = list(si.on_wait) if si is not None and si.on_wait else []
                if len(waits) > limit:
                    keep = waits[-limit:]
                    for w in waits[:-limit]:
                        nop = mybir.InstNoOp(
                            name=f"WC-{k}", ins=[], outs=[], engine=ins.engine
                        )
                        k += 1
                        nop.sync_info = mybir.SyncInfo(on_wait=[w], on_update=[])
                        out.append(nop)
                    si.on_wait = keep
                out.append(ins)
            bb.instructions[:] = out
    return k


def build_nc():
    nc = bass.Bass()
    X = nc.declare_dram_parameter("x", [NN_PER_CORE, C, T, VV], F16, isOutput=False)
    WQK = nc.declare_dram_parameter("wqkT", [C, 2 * C], F16, isOutput=False)
    WV = nc.declare_dram_parameter("wvT", [C, C], F16, isOutput=False)
    WP = nc.declare_dram_parameter("wprojT", [C, C], F16, isOutput=False)
    Y = nc.declare_dram_parameter("y", [NN_PER_CORE, C, T, VV], F32, isOutput=True)

    with tile.TileContext(nc) as tc:
        with (
            tc.tile_pool(name="consts", bufs=1) as consts,
            tc.tile_pool(name="perg", bufs=2) as perg,
            tc.tile_pool(name="pers", bufs=6) as pers,
            tc.tile_pool(name="pbig", bufs=2, space="PSUM") as pbig,
            tc.tile_pool(name="psmall", bufs=1, space="PSUM") as psmall,
        ):
            # ---- load + fp32r-convert the weights (DVE produces every
            # matmul operand so each matmul carries a single wait) ----
            wqk_r, wv_r, wp_r = [], [], []
            for kc in range(4):
                r0 = consts.tile([128, 2 * C], F16, tag=f"wqkr{kc}")
                nc.sync.dma_start(out=r0, in_=WQK[kc * 128:(kc + 1) * 128, :])
                wqk_r.append(r0)
                r1 = consts.tile([128, C], F16, tag=f"wvr{kc}")
                nc.sync.dma_start(out=r1, in_=WV[kc * 128:(kc + 1) * 128, :])
                wv_r.append(r1)
                r2 = consts.tile([128, C], F16, tag=f"wpr{kc}")
                nc.sync.dma_start(out=r2, in_=WP[kc * 128:(kc + 1) * 128, :])
                wp_r.append(r2)

            for g in range(NGROUPS):
                nn = g // (T // TG)
                t0 = (g % (T // TG)) * TG

                # ---- load x slab: 4 c-chunks of [128, 16, 25] ----
                xp = []
                for kc in range(4):
                    xq = perg.tile([128, TG, 32], F16, tag=f"xp{kc}")
                    nc.sync.dma_start(
                        out=xq[:, :, 0:VV],
                        in_=X[nn, kc * 128:(kc + 1) * 128, t0:t0 + TG, :],
                    )
                    xp.append(xq)

                # ---- q^T / k^T: out chunk m rows = c' = h*64+d (heads
                # 2m, 2m+1), cols = (b, i);  m 0-3 = q^T, 4-7 = k^T ----
                qkT = []
                for m in range(8):
                    pq = pbig.tile([128, NB], F32, tag="big")
                    for kc in range(4):
                        nc.tensor.matmul(
                            pq[:],
                            wqk_r[kc][:, m * 128:(m + 1) * 128],
                            xp[kc][:, :, 0:VV],
                            start=(kc == 0), stop=(kc == 3),
                        )
                    qc = perg.tile([128, NB], F16, tag=f"qkT{m}")
                    nc.vector.tensor_copy(qc[:, 0:NB // 2], pq[:, 0:NB // 2])
                    nc.vector.tensor_copy(qc[:, NB // 2:], pq[:, NB // 2:])
                    qkT.append(qc)

                oT = perg.tile([128, 4, NB], F16, tag="oT", name="oT")

                for sub in range(4):
                    bcol0 = sub * 4 * VV

                    pv = pbig.tile([128, C], F32, tag="big", name="pv")
                    for kc in range(4):
                        nc.tensor.matmul(
                            pv[:],
                            xp[kc][:, sub * 4:sub * 4 + 4, :],
                            wv_r[kc][:],
                            start=(kc == 0), stop=(kc == 3),
                        )
                    v2 = [pers.tile([64, C], F16, tag=f"v2{q}", name=f"v2{q}") for q in range(2)]
                    for q in range(2):
                        nc.scalar.activation(
                            v2[q][:, :], pv[q * 64:(q + 1) * 64, :],
                            mybir.ActivationFunctionType.Copy,
                        )

                    psm = [
                        psmall.tile([128, 4, VV], F32, tag=f"psm{par}", name=f"psm{par}", bufs=2)
                        for par in range(2)
                    ]
                    for h in range(H):
                        m, par, r0 = h // 2, h % 2, (h % 2) * 64
                        for b4 in range(4):
                            bcol = bcol0 + b4 * VV
                            nc.tensor.matmul(
                                psm[par][b4 * 32:b4 * 32 + 25, m, :],
                                qkT[4 + m][r0:r0 + 64, bcol:bcol + VV],
                                qkT[m][r0:r0 + 64, bcol:bcol + VV],
                                start=True, stop=True,
                                tile_position=(r0, b4 * 32),
                            )

                    e_t = perg.tile([128, VV, H], F32, tag="e_t", bufs=3)
                    for par in range(2):
                        nc.scalar.activation(
                            e_t[:, :, par::2],
                            psm[par][:].rearrange("p m i -> p i m"),
                            mybir.ActivationFunctionType.Exp,
                        )
                    D = perg.tile([128, VV], F32, tag="D", bufs=3)
                    nc.vector.reduce_sum(out=D[:], in_=e_t[:], axis=mybir.AxisListType.X)
                    rD = perg.tile([128, VV], F32, tag="rD", bufs=3)
                    nc.vector.reciprocal(rD[:], D[:])
                    p2 = [pers.tile([64, VV, H], F16, tag=f"p2{q}", name=f"p2{q}") for q in range(2)]
                    for q in range(2):
                        nc.vector.tensor_mul(
                            p2[q][:],
                            e_t[q * 64:(q + 1) * 64, :, :],
                            rD[q * 64:(q + 1) * 64, :]
                            .unsqueeze(2).broadcast_to([64, VV, H]),
                        )

                    po = [
                        psmall.tile([128, 4, 2 * VV], F32, tag=f"po{e}", name=f"po{e}")
                        for e in range(2)
                    ]
                    for b4 in range(4):
                        q, e = b4 // 2, b4 % 2
                        for h in range(H):
                            m, c0 = h // 2, (h % 2) * 64
                            nc.tensor.matmul(
                                po[e][c0:c0 + 64, m, q * VV:(q + 1) * VV],
                                v2[q][e * 32:e * 32 + 25, h * HD:(h + 1) * HD],
                                p2[q][e * 32:e * 32 + 25, :, h],
                                start=True, stop=True,
                                tile_position=(e * 32, c0),
                            )
                    for e in range(2):
                        dst = oT[:].rearrange(
                            "p m (b i) -> p m b i", i=VV
                        )[:, :, sub * 4 + e:sub * 4 + e + 3:2, :]
                        nc.vector.tensor_copy(
                            dst, po[e][:].rearrange(
                                "p m (b i) -> p m b i", i=VV
                            )
                        )

                # ---- proj: final^T[co, (b,i)] ----
                for co in range(4):
                    pf = pbig.tile([128, NB], F32, tag="big")
                    for kc in range(4):
                        nc.tensor.matmul(
                            pf[:],
                            wp_r[kc][:, co * 128:(co + 1) * 128],
                            oT[:, kc, :],
                            start=(kc == 0), stop=(kc == 3),
                        )
                    fin = perg.tile([128, NB], F32, tag=f"fin{co}")
                    nc.scalar.activation(
                        fin[:], pf[:], mybir.ActivationFunctionType.Copy,
                    )
                    nc.sync.dma_start(
                        out=Y[nn, co * 128:(co + 1) * 128, t0:t0 + TG, :],
                        in_=fin[:].rearrange("p (t v) -> p t v", t=TG),
                    )
    return nc


LAST_RESULT = {}


def kernel(x: np.ndarray, w_qkv: np.ndarray, w_proj: np.ndarray,
           _trace: bool = False) -> np.ndarray:
    n, c, t, vv = x.shape
    assert (n, c, t, vv) == (16, 512, 256, 25)
    scale = np.float32((c // H) ** -0.5)

    wq = w_qkv[:c] * scale
    wk = w_qkv[c:2 * c]
    wv = w_qkv[2 * c:]
    wqkT = np.ascontiguousarray(np.concatenate([wq, wk], axis=0).T.astype(np.float16))
    wvT = np.ascontiguousarray(wv.T.astype(np.float16))
    wprojT = np.ascontiguousarray(w_proj.T.astype(np.float16))

    nc = build_nc()
    split_excess_waits(nc)
    in_maps = []
    for core in range(N_CORES):
        shard = np.ascontiguousarray(
            x[core * NN_PER_CORE:(core + 1) * NN_PER_CORE].astype(np.float16)
        )
        in_maps.append({"x": shard, "wqkT": wqkT, "wvT": wvT, "wprojT": wprojT})

    kw = {}
    if _trace:
        import tempfile
        kw = dict(trace=True, tmpdir=tempfile.mkdtemp(prefix="attn2_trace_"))
    res = run_bass_kernel_spmd(nc, in_maps, list(range(N_CORES)), **kw)
    LAST_RESULT["res"] = res
    LAST_RESULT["tmpdir"] = kw.get("tmpdir")
    out = np.empty((n, c, t, vv), dtype=np.float32)
    for core in range(N_CORES):
        out[core * NN_PER_CORE:(core + 1) * NN_PER_CORE] = res.results[core]["y"]
    return out



# revision 4
# speedup vs baseline: 1.2865x; 1.2865x over previous
"""Trainium2 Bass kernel for nn_Attention2 (dense transformer block with
softmax over the heads axis).

Computation per (n, t) batch b (B = n*t = 4096 total, X_b = x[n,:,t,:].T is
[vv=25, c=512]):
    qkv = X_b @ w_qkv.T, split into q,k,v heads [h=8, 25, hd=64]
    s[h,i,j] = (q[h,i,:] . k[h,j,:]) / 8      (scale folded into w_q on host)
    p = softmax over h (axis 0)
    o[h,i,:] = sum_j p[h,i,j] v[h,j,:]  -> [25, 512] -> @ w_proj.T
    out[n,:,t,:] = result.T

Sharding: data-parallel over n, 2 n-values (512 batches) per core, 8 cores.

v2: software-pipelined across groups.  The v1 kernel ran group-serial: the
scores->exp->reduce->recip->mul->attnv dependency chain left the PE idle
~6us per group, re-throttling the HAM clock gate every group (42% of the
kernel ran at 1.2GHz).  v2 interleaves group g+1's qkv GEMMs into group g's
attention core as PE filler (emission order = scheduler priority), deepens
the big-PSUM rotation to 4 banks, slims the softmax to
exp(ACT)->strided-reduce->recip->mul (all fp16), merges PSUM evacuations
into fewer/bigger ops balanced across DVE and ACT, outputs fp16 (cast to
fp32 on host), and warms the PE with dummy matmuls during the initial
weight DMA.
"""
import numpy as np
import concourse.bass as bass
import concourse.mybir as mybir
import concourse.tile as tile
from concourse.bass_utils import run_bass_kernel_spmd
from concourse.vector_clock import ScopedClock, VectorClock

F32 = mybir.dt.float32
F16 = mybir.dt.float16

N_CORES = 8
NN_PER_CORE = 2        # n values per core
T = 256
VV = 25
C = 512
H = 8
HD = 64
TG = 16                # t values (batches) per group
NGROUPS = NN_PER_CORE * (T // TG)   # 32 groups per core
NB = TG * VV           # 400 moving columns per group
NSUB = 4               # sub-blocks of 4 batches per group


def _split_drain_and_barrier(self, tick_clock, wait_clock):
    # walrus caps sync-wait commands at 1 for CTRL_NO; split the kernel-tail
    # drain into one drain per pending proc.
    vc = tick_clock.global_clock
    n = len(vc)
    for i in range(n):
        if vc[i] == 0:
            continue
        sub = VectorClock([vc[j] if j == i else 0 for j in range(n)])
        d = self.nc.sync.drain()
        wait_clock.add_sem_waits(d.ins, ScopedClock({None: sub}))
    self.nc.all_engine_barrier()
    assert self.sems is not None
    popped = self.nc._tile_sem_poison_stack.pop()
    assert popped is self._sem_poison
    self.nc.clear_and_free_semaphores(list(self.sems.allocated().values()))
    self.nc.all_engine_barrier()


tile.TileContext._drain_and_barrier = _split_drain_and_barrier


def split_excess_waits(nc, limit=1):
    """walrus codegen allows very few sync-wait commands per instruction
    (1 for matmul/drain/DMA structs).  Move excess waits onto same-engine
    NoOp carriers inserted just before the instruction — same semantics,
    since each engine executes its queue in order."""
    k = 0
    for fn in nc.m.functions:
        for bb in fn.blocks:
            out = []
            for ins in bb.instructions:
                si = ins.sync_info
                waits = list(si.on_wait) if si is not None and si.on_wait else []
                if len(waits) > limit:
                    keep = waits[-limit:]
                    for w in waits[:-limit]:
                        nop = mybir.InstNoOp(
                            name=f"WC-{k}", ins=[], outs=[], engine=ins.engine
                        )
                        k += 1
                        nop.sync_info = mybir.SyncInfo(on_wait=[w], on_update=[])
                        out.append(nop)
                    si.on_wait = keep
                out.append(ins)
            bb.instructions[:] = out
    return k


def build_nc():
    nc = bass.Bass()
    X = nc.declare_dram_parameter("x", [NN_PER_CORE, C, T, VV], F16, isOutput=False)
    WQK = nc.declare_dram_parameter("wqkT", [C, 2 * C], F16, isOutput=False)
    WV = nc.declare_dram_parameter("wvT", [C, C], F16, isOutput=False)
    WP = nc.declare_dram_parameter("wprojT", [C, C], F16, isOutput=False)
    Y = nc.declare_dram_parameter("y", [NN_PER_CORE, C, T, VV], F16, isOutput=True)

    with tile.TileContext(nc) as tc:
        with (
            tc.tile_pool(name="consts", bufs=1) as consts,
            tc.tile_pool(name="xpool", bufs=3) as xpool,
            tc.tile_pool(name="qpool", bufs=2) as qpool,
            tc.tile_pool(name="vpool", bufs=2) as vpool,
            tc.tile_pool(name="smpool", bufs=2) as smpool,
            tc.tile_pool(name="opool", bufs=2) as opool,
            tc.tile_pool(name="fpool", bufs=2) as fpool,
            tc.tile_pool(name="pbig", bufs=4, space="PSUM") as pbig,
            tc.tile_pool(name="psmall", bufs=1, space="PSUM") as psmall,
        ):
            # ---- weight loads ----
            wqk_r, wv_r, wp_r = [], [], []
            for kc in range(4):
                r0 = consts.tile([128, 2 * C], F16, tag=f"wqkr{kc}")
                nc.sync.dma_start(out=r0, in_=WQK[kc * 128:(kc + 1) * 128, :])
                wqk_r.append(r0)
                r1 = consts.tile([128, C], F16, tag=f"wvr{kc}")
                nc.sync.dma_start(out=r1, in_=WV[kc * 128:(kc + 1) * 128, :])
                wv_r.append(r1)
                r2 = consts.tile([128, C], F16, tag=f"wpr{kc}")
                nc.sync.dma_start(out=r2, in_=WP[kc * 128:(kc + 1) * 128, :])
                wp_r.append(r2)

            # ---- PE warmup: ~5us of dummy matmuls overlapping the weight
            # DMAs so the HAM clock gate reaches 8/8 before group 0 ----
            wu = consts.tile([128, C], F16, tag="warm")
            nc.vector.memset(wu[:], 0.0)
            for _ in range(12):
                pwu = pbig.tile([128, C], F32, tag="big", name="pwu")
                nc.tensor.matmul(pwu[:], wu[:, 0:128], wu[:],
                                 start=True, stop=True)

            def load_x(g):
                xq = []
                nn = g // (T // TG)
                t0 = (g % (T // TG)) * TG
                for kc in range(4):
                    xt = xpool.tile([128, TG, 32], F16, tag=f"xp{kc}", name="xt")
                    nc.sync.dma_start(
                        out=xt[:, :, 0:VV],
                        in_=X[nn, kc * 128:(kc + 1) * 128, t0:t0 + TG, :],
                    )
                    xq.append(xt)
                return xq

            def emit_qk_chunk(m, xp, qc_next):
                # q^T/k^T chunk m: c'-rows m*128..m*128+128, cols = (b, i)
                pq = pbig.tile([128, NB], F32, tag="big", name="pq")
                for kc in range(4):
                    nc.tensor.matmul(
                        pq[:],
                        wqk_r[kc][:, m * 128:(m + 1) * 128],
                        xp[kc][:, :, 0:VV],
                        start=(kc == 0), stop=(kc == 3),
                    )
                qcm = qpool.tile([128, NB], F16, tag=f"qkT{m}", name="qcm")
                if (m % 4) < 2:
                    nc.vector.tensor_copy(qcm[:], pq[:])
                else:
                    nc.scalar.activation(
                        qcm[:], pq[:], mybir.ActivationFunctionType.Copy
                    )
                qc_next[m] = qcm

            def emit_v_sub(s, xp, v2_next):
                # v for batches s*4..s*4+4 in [token, c'] layout
                pv = pbig.tile([128, C], F32, tag="big", name="pv")
                for kc in range(4):
                    nc.tensor.matmul(
                        pv[:],
                        xp[kc][:, s * 4:s * 4 + 4, :],
                        wv_r[kc][:],
                        start=(kc == 0), stop=(kc == 3),
                    )
                pair = []
                for q in range(2):
                    v2 = vpool.tile([64, C], F16, tag=f"v2_{s}_{q}", name="v2")
                    nc.scalar.activation(
                        v2[:, :], pv[q * 64:(q + 1) * 64, :],
                        mybir.ActivationFunctionType.Copy,
                    )
                    pair.append(v2)
                v2_next[s] = pair

            def emit_scores(s, qc, psm):
                for h in range(H):
                    m, par, r0 = h // 2, h % 2, (h % 2) * 64
                    for b4 in range(4):
                        bcol = (s * 4 + b4) * VV
                        nc.tensor.matmul(
                            psm[par][b4 * 32:b4 * 32 + 25, m, :],
                            qc[4 + m][r0:r0 + 64, bcol:bcol + VV],
                            qc[m][r0:r0 + 64, bcol:bcol + VV],
                            start=True, stop=True,
                            tile_position=(r0, b4 * 32),
                        )

            def emit_attnv(s, v2pair, p2q, po):
                for b4 in range(4):
                    q, e = b4 // 2, b4 % 2
                    for h in range(H):
                        m, c0 = h // 2, (h % 2) * 64
                        nc.tensor.matmul(
                            po[e][c0:c0 + 64, m, q * VV:(q + 1) * VV],
                            v2pair[q][e * 32:e * 32 + 25, h * HD:(h + 1) * HD],
                            p2q[q][e * 32:e * 32 + 25, h % 2, h // 2, :],
                            start=True, stop=True,
                            tile_position=(e * 32, c0),
                        )

            # ---- prologue: x(0), x(1), qkv(0) ----
            xp_cur = load_x(0)          # consumed by group 0 (already emitted qkv)
            qc_cur = [None] * 8
            v2_cur = [None] * NSUB
            for s in range(NSUB):
                emit_qk_chunk(2 * s, xp_cur, qc_cur)
                emit_qk_chunk(2 * s + 1, xp_cur, qc_cur)
                emit_v_sub(s, xp_cur, v2_cur)
            xp_a = load_x(1)            # for qkv(1) emitted inside iteration 0

            # ---- main pipelined loop ----
            for g in range(NGROUPS):
                nn = g // (T // TG)
                t0 = (g % (T // TG)) * TG
                have_next = g + 1 < NGROUPS

                if g + 2 < NGROUPS:
                    xp_b = load_x(g + 2)
                else:
                    xp_b = None
                qc_next = [None] * 8
                v2_next = [None] * NSUB
                oT = opool.tile([128, 4, NB], F16, tag="oT", name="oT")

                for s in range(NSUB):
                    # scores wave for (g, s)
                    psm = [
                        psmall.tile([128, 4, VV], F32, tag=f"psm{par}",
                                    name=f"psm{par}",
                                    padded_shape=[128, 4, 128])
                        for par in range(2)
                    ]
                    emit_scores(s, qc_cur, psm)

                    # softmax over heads (axis split across psm[0]/psm[1])
                    e_t = smpool.tile([128, 2, 4, VV], F16, tag=f"e{s}", name="e_t")
                    for par in range(2):
                        nc.scalar.activation(
                            e_t[:, par], psm[par][:],
                            mybir.ActivationFunctionType.Exp,
                        )

                    # PE filler: next group's qk chunk 2s
                    if have_next:
                        emit_qk_chunk(2 * s, xp_a, qc_next)

                    D = smpool.tile([128, VV], F32, tag=f"D{s}", name="D")
                    nc.vector.reduce_sum(
                        out=D[:],
                        in_=e_t[:].rearrange("p a m i -> p i (a m)"),
                        axis=mybir.AxisListType.X,
                    )
                    rD = smpool.tile([128, VV], F16, tag=f"rD{s}", name="rD")
                    with nc.allow_low_precision(reason="1/D in fp16: D in [2e-2, 3e3], rel err ~5e-4 vs 2e-2 budget"):
                        nc.vector.reciprocal(rD[:], D[:])
                    p2q = []
                    for q in range(2):
                        p2 = smpool.tile([64, 2, 4, VV], F16, tag=f"p2_{s}_{q}",
                                         name="p2")
                        nc.vector.tensor_mul(
                            p2[:],
                            e_t[q * 64:(q + 1) * 64],
                            rD[q * 64:(q + 1) * 64]
                            .unsqueeze(1).unsqueeze(1)
                            .broadcast_to([64, 2, 4, VV]),
                        )
                        p2q.append(p2)

                    # PE filler: next group's qk chunk 2s+1
                    if have_next:
                        emit_qk_chunk(2 * s + 1, xp_a, qc_next)

                    # attention @ v wave for (g, s)
                    po = [
                        psmall.tile([128, 4, 2 * VV], F32, tag=f"po{e}",
                                    name=f"po{e}",
                                    padded_shape=[128, 4, 128])
                        for e in range(2)
                    ]
                    emit_attnv(s, v2_cur[s], p2q, po)

                    # evacuate po -> oT (fp16)
                    for e in range(2):
                        dst = oT[:].rearrange(
                            "p m (b i) -> p m b i", i=VV
                        )[:, :, s * 4 + e:s * 4 + e + 3:2, :]
                        nc.vector.tensor_copy(
                            dst, po[e][:].rearrange("p m (b i) -> p m b i", i=VV)
                        )

                    # PE filler: next group's v for sub s
                    if have_next:
                        emit_v_sub(s, xp_a, v2_next)

                # proj + store for group g
                for co in range(4):
                    pf = pbig.tile([128, NB], F32, tag="big", name="pf")
                    for kc in range(4):
                        nc.tensor.matmul(
                            pf[:],
                            wp_r[kc][:, co * 128:(co + 1) * 128],
                            oT[:, kc, :],
                            start=(kc == 0), stop=(kc == 3),
                        )
                    fin = fpool.tile([128, NB], F16, tag=f"fin{co}", name="fin")
                    if co < 2:
                        nc.vector.tensor_copy(fin[:], pf[:])
                    else:
                        nc.scalar.activation(
                            fin[:], pf[:], mybir.ActivationFunctionType.Copy
                        )
                    nc.sync.dma_start(
                        out=Y[nn, co * 128:(co + 1) * 128, t0:t0 + TG, :],
                        in_=fin[:].rearrange("p (t v) -> p t v", t=TG),
                    )

                qc_cur = qc_next
                v2_cur = v2_next
                xp_cur = xp_a
                xp_a = xp_b
    return nc


LAST_RESULT = {}


def kernel(x: np.ndarray, w_qkv: np.ndarray, w_proj: np.ndarray,
           _trace: bool = False) -> np.ndarray:
    n, c, t, vv = x.shape
    assert (n, c, t, vv) == (16, 512, 256, 25)
    scale = np.float32((c // H) ** -0.5)

    wq = w_qkv[:c] * scale
    wk = w_qkv[c:2 * c]
    wv = w_qkv[2 * c:]
    wqkT = np.ascontiguousarray(np.concatenate([wq, wk], axis=0).T.astype(np.float16))
    wvT = np.ascontiguousarray(wv.T.astype(np.float16))
    wprojT = np.ascontiguousarray(w_proj.T.astype(np.float16))

    nc = build_nc()
    split_excess_waits(nc)
    in_maps = []
    for core in range(N_CORES):
        shard = np.ascontiguousarray(
            x[core * NN_PER_CORE:(core + 1) * NN_PER_CORE].astype(np.float16)
        )
        in_maps.append({"x": shard, "wqkT": wqkT, "wvT": wvT, "wprojT": wprojT})

    kw = {}
    if _trace:
        import tempfile
        kw = dict(trace=True, tmpdir=tempfile.mkdtemp(prefix="attn2_trace_"))
    res = run_bass_kernel_spmd(nc, in_maps, list(range(N_CORES)), **kw)
    LAST_RESULT["res"] = res
    LAST_RESULT["tmpdir"] = kw.get("tmpdir")
    out = np.empty((n, c, t, vv), dtype=np.float32)
    for core in range(N_CORES):
        out[core * NN_PER_CORE:(core + 1) * NN_PER_CORE] = \
            res.results[core]["y"].astype(np.float32)
    return out


# revision 8
# speedup vs baseline: 1.4816x; 1.1516x over previous
"""Trainium2 Bass kernel for nn_Attention2 (dense transformer block with
softmax over the heads axis).

Computation per (n, t) batch b (B = n*t = 4096 total, X_b = x[n,:,t,:].T is
[vv=25, c=512]):
    qkv = X_b @ w_qkv.T, split into q,k,v heads [h=8, 25, hd=64]
    s[h,i,j] = (q[h,i,:] . k[h,j,:]) / 8      (scale folded into w_q on host)
    p = softmax over h (axis 0)
    o[h,i,:] = sum_j p[h,i,j] v[h,j,:]  -> [25, 512] -> @ w_proj.T
    out[n,:,t,:] = result.T

Sharding: data-parallel over n, 2 n-values (512 batches) per core, 8 cores.

v2: software-pipelined across groups.  The v1 kernel ran group-serial: the
scores->exp->reduce->recip->mul->attnv dependency chain left the PE idle
~6us per group, re-throttling the HAM clock gate every group (42% of the
kernel ran at 1.2GHz).  v2 interleaves group g+1's qkv GEMMs into group g's
attention core as PE filler (emission order = scheduler priority), deepens
the big-PSUM rotation to 4 banks, slims the softmax to
exp(ACT)->strided-reduce->recip->mul (all fp16), merges PSUM evacuations
into fewer/bigger ops balanced across DVE and ACT, outputs fp16 (cast to
fp32 on host), and warms the PE with dummy matmuls during the initial
weight DMA.
"""
import numpy as np
import concourse.bass as bass
import concourse.mybir as mybir
import concourse.tile as tile
from concourse.bass_utils import run_bass_kernel_spmd
from concourse.vector_clock import ScopedClock, VectorClock

F32 = mybir.dt.float32
F16 = mybir.dt.float16

N_CORES = 8
NN_PER_CORE = 2        # n values per core
T = 256
VV = 25
C = 512
H = 8
HD = 64
TG = 16                # t values (batches) per group
NGROUPS = NN_PER_CORE * (T // TG)   # 32 groups per core
NB = TG * VV           # 400 moving columns per group
NSUB = 4               # sub-blocks of 4 batches per group


def _split_drain_and_barrier(self, tick_clock, wait_clock):
    # walrus caps sync-wait commands at 1 for CTRL_NO; split the kernel-tail
    # drain into one drain per pending proc.
    vc = tick_clock.global_clock
    n = len(vc)
    for i in range(n):
        if vc[i] == 0:
            continue
        sub = VectorClock([vc[j] if j == i else 0 for j in range(n)])
        d = self.nc.sync.drain()
        wait_clock.add_sem_waits(d.ins, ScopedClock({None: sub}))
    self.nc.all_engine_barrier()
    assert self.sems is not None
    popped = self.nc._tile_sem_poison_stack.pop()
    assert popped is self._sem_poison
    self.nc.clear_and_free_semaphores(list(self.sems.allocated().values()))
    self.nc.all_engine_barrier()


tile.TileContext._drain_and_barrier = _split_drain_and_barrier


def split_excess_waits(nc, limit=1):
    """walrus codegen allows very few sync-wait commands per instruction
    (1 for matmul/drain/DMA structs).  Move excess waits onto same-engine
    NoOp carriers inserted just before the instruction — same semantics,
    since each engine executes its queue in order."""
    k = 0
    for fn in nc.m.functions:
        for bb in fn.blocks:
            out = []
            for ins in bb.instructions:
                si = ins.sync_info
                waits = list(si.on_wait) if si is not None and si.on_wait else []
                if len(waits) > limit:
                    keep = waits[-limit:]
                    for w in waits[:-limit]:
                        nop = mybir.InstNoOp(
                            name=f"WC-{k}", ins=[], outs=[], engine=ins.engine
                        )
                        k += 1
                        nop.sync_info = mybir.SyncInfo(on_wait=[w], on_update=[])
                        out.append(nop)
                    si.on_wait = keep
                out.append(ins)
            bb.instructions[:] = out
    return k


def build_nc():
    nc = bass.Bass()
    X = nc.declare_dram_parameter("x", [NN_PER_CORE, C, T, VV], F16, isOutput=False)
    WQK = nc.declare_dram_parameter("wqkT", [C, 2 * C], F16, isOutput=False)
    WV = nc.declare_dram_parameter("wvT", [C, C], F16, isOutput=False)
    WP = nc.declare_dram_parameter("wprojT", [C, C], F16, isOutput=False)
    Y = nc.declare_dram_parameter("y", [NN_PER_CORE, C, T, VV], F16, isOutput=True)

    with tile.TileContext(nc) as tc:
        with (
            tc.tile_pool(name="consts", bufs=1) as consts,
            tc.tile_pool(name="xpool", bufs=3) as xpool,
            tc.tile_pool(name="qpool", bufs=2) as qpool,
            tc.tile_pool(name="vpool", bufs=2) as vpool,
            tc.tile_pool(name="smpool", bufs=2) as smpool,
            tc.tile_pool(name="opool", bufs=2) as opool,
            tc.tile_pool(name="fpool", bufs=2) as fpool,
            tc.tile_pool(name="pbig", bufs=4, space="PSUM") as pbig,
            tc.tile_pool(name="psmall", bufs=1, space="PSUM") as psmall,
        ):
            # ---- weight loads (wqk first: the first qk chunk needs all 4) ----
            wqk_r, wv_r, wp_r = [], [], []
            for kc in range(4):
                r0 = consts.tile([128, 2 * C], F16, tag=f"wqkr{kc}")
                nc.sync.dma_start(out=r0, in_=WQK[kc * 128:(kc + 1) * 128, :])
                wqk_r.append(r0)

            # ---- PE warmup: ~10us of dummy matmuls overlapping the weight
            # and x(0) DMAs so the HAM clock gate reaches 8/8 before group 0 ----
            wu = consts.tile([128, C], F16, tag="warm")
            nc.vector.memset(wu[:], 0.0)
            for _ in range(20):
                pwu = pbig.tile([128, C], F32, tag="big", name="pwu")
                nc.tensor.matmul(pwu[:], wu[:, 0:128], wu[:],
                                 start=True, stop=True)

            def load_x(g):
                xq = []
                nn = g // (T // TG)
                t0 = (g % (T // TG)) * TG
                for kc in range(4):
                    xt = xpool.tile([128, TG, 32], F16, tag=f"xp{kc}", name="xt")
                    nc.sync.dma_start(
                        out=xt[:, :, 0:VV],
                        in_=X[nn, kc * 128:(kc + 1) * 128, t0:t0 + TG, :],
                    )
                    xq.append(xt)
                return xq

            def emit_qk_chunk(m, xp, qc_next):
                # q^T/k^T chunk m: c'-rows m*128..m*128+128, cols = (b, i)
                pq = pbig.tile([128, NB], F32, tag="big", name="pq")
                for kc in range(4):
                    nc.tensor.matmul(
                        pq[:],
                        wqk_r[kc][:, m * 128:(m + 1) * 128],
                        xp[kc][:, :, 0:VV],
                        start=(kc == 0), stop=(kc == 3),
                    )
                # evac engine alternates by m parity so every sub's two qk
                # chunks split 1 ACT + 1 DVE; score MMs for head-pair hp read
                # qc[hp] and qc[hp+4] (same parity -> same producing engine,
                # keeping each matmul at a single cross-engine wait)
                qcm = qpool.tile([128, NB], F16, tag=f"qkT{m}", name="qcm")
                if m % 2 == 0:
                    nc.scalar.activation(
                        qcm[:], pq[:], mybir.ActivationFunctionType.Copy
                    )
                else:
                    nc.vector.tensor_copy(qcm[:], pq[:])
                qc_next[m] = qcm

            def emit_v_sub(s, xp, v2_next):
                # v for batches s*4..s*4+4 in [token, c'] layout
                pv = pbig.tile([128, C], F32, tag="big", name="pv")
                for kc in range(4):
                    nc.tensor.matmul(
                        pv[:],
                        xp[kc][:, s * 4:s * 4 + 4, :],
                        wv_r[kc][:],
                        start=(kc == 0), stop=(kc == 3),
                    )
                pair = []
                for q in range(2):
                    v2 = vpool.tile([64, C], F16, tag=f"v2_{s}_{q}", name="v2")
                    nc.scalar.activation(
                        v2[:, :], pv[q * 64:(q + 1) * 64, :],
                        mybir.ActivationFunctionType.Copy,
                    )
                    pair.append(v2)
                v2_next[s] = pair

            def emit_scores(s, qc, psm):
                for h in range(H):
                    m, par, r0 = h // 2, h % 2, (h % 2) * 64
                    for b4 in range(4):
                        bcol = (s * 4 + b4) * VV
                        nc.tensor.matmul(
                            psm[par][b4 * 32:b4 * 32 + 25, m, :],
                            qc[4 + m][r0:r0 + 64, bcol:bcol + VV],
                            qc[m][r0:r0 + 64, bcol:bcol + VV],
                            start=True, stop=True,
                            tile_position=(r0, b4 * 32),
                        )

            def emit_attnv(s, v2pair, p2q, po):
                for b4 in range(4):
                    q, e = b4 // 2, b4 % 2
                    for h in range(H):
                        m, c0 = h // 2, (h % 2) * 64
                        nc.tensor.matmul(
                            po[e][c0:c0 + 64, m, q * VV:(q + 1) * VV],
                            v2pair[q][e * 32:e * 32 + 25, h * HD:(h + 1) * HD],
                            p2q[q][e * 32:e * 32 + 25, h % 2, h // 2, :],
                            start=True, stop=True,
                            tile_position=(e * 32, c0),
                        )

            # ---- prologue: x(0), remaining weights, qkv(0), x(1) ----
            xp_cur = load_x(0)          # consumed by group 0 (already emitted qkv)
            for kc in range(4):
                r1 = consts.tile([128, C], F16, tag=f"wvr{kc}")
                nc.sync.dma_start(out=r1, in_=WV[kc * 128:(kc + 1) * 128, :])
                wv_r.append(r1)
                r2 = consts.tile([128, C], F16, tag=f"wpr{kc}")
                nc.sync.dma_start(out=r2, in_=WP[kc * 128:(kc + 1) * 128, :])
                wp_r.append(r2)
            qc_cur = [None] * 8
            v2_cur = [None] * NSUB
            for s in range(NSUB):
                emit_qk_chunk(2 * s, xp_cur, qc_cur)
                emit_qk_chunk(2 * s + 1, xp_cur, qc_cur)
                emit_v_sub(s, xp_cur, v2_cur)
            xp_a = load_x(1)            # for qkv(1) emitted inside iteration 0

            # ---- main pipelined loop ----
            for g in range(NGROUPS):
                nn = g // (T // TG)
                t0 = (g % (T // TG)) * TG
                have_next = g + 1 < NGROUPS

                if g + 2 < NGROUPS:
                    xp_b = load_x(g + 2)
                else:
                    xp_b = None
                qc_next = [None] * 8
                v2_next = [None] * NSUB
                oT = opool.tile([128, 4, NB], F16, tag="oT", name="oT")

                for s in range(NSUB):
                    # scores wave for (g, s)
                    psm = [
                        psmall.tile([128, 4, VV], F32, tag=f"psm{par}",
                                    name=f"psm{par}",
                                    padded_shape=[128, 4, 128])
                        for par in range(2)
                    ]
                    emit_scores(s, qc_cur, psm)

                    # softmax over heads (axis split across psm[0]/psm[1])
                    e_t = smpool.tile([128, 2, 4, VV], F16, tag=f"e{s}", name="e_t")
                    for par in range(2):
                        nc.scalar.activation(
                            e_t[:, par], psm[par][:],
                            mybir.ActivationFunctionType.Exp,
                        )

                    # PE filler: next group's qk chunk 2s
                    if have_next:
                        emit_qk_chunk(2 * s, xp_a, qc_next)

                    D = smpool.tile([128, VV], F32, tag=f"D{s}", name="D")
                    nc.vector.reduce_sum(
                        out=D[:],
                        in_=e_t[:].rearrange("p a m i -> p i (a m)"),
                        axis=mybir.AxisListType.X,
                    )
                    rD = smpool.tile([128, VV], F16, tag=f"rD{s}", name="rD")
                    with nc.allow_low_precision(reason="1/D in fp16: D in [2e-2, 3e3], rel err ~5e-4 vs 2e-2 budget"):
                        nc.vector.reciprocal(rD[:], D[:])
                    p2q = []
                    for q in range(2):
                        p2 = smpool.tile([64, 2, 4, VV], F16, tag=f"p2_{s}_{q}",
                                         name="p2")
                        nc.vector.tensor_mul(
                            p2[:],
                            e_t[q * 64:(q + 1) * 64],
                            rD[q * 64:(q + 1) * 64]
                            .unsqueeze(1).unsqueeze(1)
                            .broadcast_to([64, 2, 4, VV]),
                        )
                        p2q.append(p2)

                    # PE filler: next group's qk chunk 2s+1
                    if have_next:
                        emit_qk_chunk(2 * s + 1, xp_a, qc_next)

                    # attention @ v wave for (g, s)
                    po = [
                        psmall.tile([128, 4, 2 * VV], F32, tag=f"po{e}",
                                    name=f"po{e}",
                                    padded_shape=[128, 4, 128])
                        for e in range(2)
                    ]
                    emit_attnv(s, v2_cur[s], p2q, po)

                    # evacuate po -> oT (fp16)
                    for e in range(2):
                        dst = oT[:].rearrange(
                            "p m (b i) -> p m b i", i=VV
                        )[:, :, s * 4 + e:s * 4 + e + 3:2, :]
                        nc.vector.tensor_copy(
                            dst, po[e][:].rearrange("p m (b i) -> p m b i", i=VV)
                        )

                    # PE filler: next group's v for sub s
                    if have_next:
                        emit_v_sub(s, xp_a, v2_next)

                # proj + store for group g.  pf rides the po banks: its WAR
                # (wait for oT evac of sub3 to release the bank) coincides
                # with proj's real data dependency on oT, so it adds no
                # serialization — and it keeps the big-pool rotation free for
                # group g+1's qkv.
                for co in range(4):
                    pf = psmall.tile([128, NB], F32, tag=f"po{co % 2}",
                                     name="pf", padded_shape=[128, 512])
                    for kc in range(4):
                        nc.tensor.matmul(
                            pf[:],
                            wp_r[kc][:, co * 128:(co + 1) * 128],
                            oT[:, kc, :],
                            start=(kc == 0), stop=(kc == 3),
                        )
                    fin = fpool.tile([128, NB], F16, tag=f"fin{co}", name="fin")
                    if co < 2:
                        nc.vector.tensor_copy(fin[:], pf[:])
                    else:
                        nc.scalar.activation(
                            fin[:], pf[:], mybir.ActivationFunctionType.Copy
                        )
                    nc.sync.dma_start(
                        out=Y[nn, co * 128:(co + 1) * 128, t0:t0 + TG, :],
                        in_=fin[:].rearrange("p (t v) -> p t v", t=TG),
                    )

                qc_cur = qc_next
                v2_cur = v2_next
                xp_cur = xp_a
                xp_a = xp_b
    return nc


LAST_RESULT = {}


def kernel(x: np.ndarray, w_qkv: np.ndarray, w_proj: np.ndarray,
           _trace: bool = False) -> np.ndarray:
    n, c, t, vv = x.shape
    assert (n, c, t, vv) == (16, 512, 256, 25)
    scale = np.float32((c // H) ** -0.5)

    wq = w_qkv[:c] * scale
    wk = w_qkv[c:2 * c]
    wv = w_qkv[2 * c:]
    wqkT = np.ascontiguousarray(np.concatenate([wq, wk], axis=0).T.astype(np.float16))
    wvT = np.ascontiguousarray(wv.T.astype(np.float16))
    wprojT = np.ascontiguousarray(w_proj.T.astype(np.float16))

    nc = build_nc()
    split_excess_waits(nc)
    in_maps = []
    for core in range(N_CORES):
        shard = np.ascontiguousarray(
            x[core * NN_PER_CORE:(core + 1) * NN_PER_CORE].astype(np.float16)
        )
        in_maps.append({"x": shard, "wqkT": wqkT, "wvT": wvT, "wprojT": wprojT})

    kw = {}
    if _trace:
        import tempfile
        kw = dict(trace=True, tmpdir=tempfile.mkdtemp(prefix="attn2_trace_"))
    res = run_bass_kernel_spmd(nc, in_maps, list(range(N_CORES)), **kw)
    LAST_RESULT["res"] = res
    LAST_RESULT["tmpdir"] = kw.get("tmpdir")
    out = np.empty((n, c, t, vv), dtype=np.float32)
    for core in range(N_CORES):
        out[core * NN_PER_CORE:(core + 1) * NN_PER_CORE] = \
            res.results[core]["y"].astype(np.float32)
    return out
